# revision 30
# baseline (speedup 1.0000x reference)
"""Trainium2 Bass kernel for nn_MoEBlock (pre-norm causal MHA + dense top-2 MoE).

Sharding: attention is head-sharded (2 of 16 heads per core) with an
AllReduce of the output-projection partials; the MoE is expert-parallel
(expert e on core e) with an AllReduce of the gate-weighted expert outputs.

Device dataflow keeps activations transposed ([feature, token]) so every
matmul contracts along the partition axis.  Matmuls run in float32r
(full PE rate for N>=512, ~2e-4 rel err) except the w2 expert matmul which
runs in bf16.  RMS norm scales are folded into the adjacent weight
matrices on the host; per-token rsqrt factors are applied via
DMA-broadcast rows.
"""

import sys

if "/opt/trn_rl_repo" not in sys.path:
    sys.path.insert(0, "/opt/trn_rl_repo")

import ml_dtypes
import numpy as np

import concourse.bacc as bacc
import concourse.mybir as mybir
import concourse.tile as tile
from concourse.bass_utils import run_bass_kernel_spmd
from concourse.masks import make_identity

# problem dims
B, S, D, H, F, E, K = 2, 2048, 1024, 16, 4096, 8, 2
HD = D // H          # 64
T = B * S            # 4096 tokens
EPS = 1e-6
N_CORES = 8
HPC = H // N_CORES   # heads per core = 2
HCOL = HPC * HD      # 128 head-dim columns per core

P = 128
QC = 512             # attention query chunk
NKT = S // P         # 16 k-tiles per batch
NQC = S // QC        # 4 q chunks per batch
ACH = 4              # attention all-reduce chunks (over tokens)
ACW = T // ACH       # 1024 tokens per AR chunk
ZC = 4               # moe token chunks
ZW = T // ZC         # 1024
NDC = D // P         # 8 d chunks
NFC = F // P         # 32 f chunks
GFC = 8              # fc per moe group
NGRP = NFC // GFC

f32 = mybir.dt.float32
f32r = mybir.dt.float32r
bf16 = mybir.dt.bfloat16
AX = mybir.AxisListType
ALU = mybir.AluOpType
ACT = mybir.ActivationFunctionType

_NC_CACHE = {}


def build_nc(debug_taps=False, sim_mode=False):
    key = (debug_taps, sim_mode)
    if key in _NC_CACHE:
        return _NC_CACHE[key]
    nc = bacc.Bacc("TRN2", target_bir_lowering=False, debug=False,
                   num_devices=1 if sim_mode else N_CORES)

    def all_reduce(src_t, dst_t):
        if sim_mode:
            # dependency-preserving stub; real AR runs on TOPSP, not our DMA
            nc.sync.dma_start(dst_t[0:1, :], src_t[0:1, :])
        else:
            nc.gpsimd.collective_compute(
                "AllReduce", ALU.add,
                replica_groups=[list(range(N_CORES))],
                ins=[src_t.opt()],
                outs=[dst_t.opt()],
            )

    # ---- I/O ----
    xT = nc.dram_tensor("xT", [D, T], f32, kind="ExternalInput")
    x_nat = nc.dram_tensor("x_nat", [T, D], f32, kind="ExternalInput")
    wq = nc.dram_tensor("wq", [D, HCOL], f32, kind="ExternalInput")
    wk = nc.dram_tensor("wk", [D, HCOL], f32, kind="ExternalInput")
    wv = nc.dram_tensor("wv", [D, HCOL], f32, kind="ExternalInput")
    wo = nc.dram_tensor("wo", [HCOL, D], f32, kind="ExternalInput")
    rw = nc.dram_tensor("rw", [D, E], f32, kind="ExternalInput")
    w1t = nc.dram_tensor("w1t", [NDC, NFC, P, P], f32, kind="ExternalInput")
    w2t = nc.dram_tensor("w2t", [NDC, NGRP, P, GFC * P], bf16, kind="ExternalInput")
    b1 = nc.dram_tensor("b1", [NFC, P], f32, kind="ExternalInput")
    b2 = nc.dram_tensor("b2", [NDC, P], f32, kind="ExternalInput")
    esel = nc.dram_tensor("esel", [1, E], f32, kind="ExternalInput")
    outT = nc.dram_tensor("outT", [D, T], f32, kind="ExternalOutput")
    taps = {}
    if debug_taps:
        taps["qT"] = nc.dram_tensor("tap_qT", [HCOL, T], f32, kind="ExternalOutput")
        taps["kT"] = nc.dram_tensor("tap_kT", [HCOL, T], f32, kind="ExternalOutput")
        taps["ctxn"] = nc.dram_tensor("tap_ctxn", [HCOL, T], f32, kind="ExternalOutput")
        taps["x1T"] = nc.dram_tensor("tap_x1T", [D, T], f32, kind="ExternalOutput")
        taps["logits"] = nc.dram_tensor("tap_logits", [T, E], f32, kind="ExternalOutput")
        taps["gates"] = nc.dram_tensor("tap_gates", [T, E], f32, kind="ExternalOutput")
        taps["r2"] = nc.dram_tensor("tap_r2", [1, T], f32, kind="ExternalOutput")

    with tile.TileContext(nc) as tc:
        with (
            tc.tile_pool(name="const", bufs=1) as cp,
            tc.tile_pool(name="dram", bufs=1, space="DRAM") as dp,
        ):
            # ---- constants ----
            ident = cp.tile([P, P], f32, tag="ident")
            make_identity(nc, ident[:])
            identr = cp.tile([P, P], f32r, tag="identr")
            nc.vector.tensor_copy(identr[:], ident[:])
            ones_r = cp.tile([P, P], f32r, tag="ones_r")
            onesf = cp.tile([P, 1], f32, tag="onesf")
            nc.gpsimd.memset(onesf[:], 1.0)
            nc.sync.dma_start(
                ones_r[:], onesf[:, 0:1].to_broadcast((P, P)).bitcast(f32r)
            )
            masks = cp.tile([P, 4 * QC], f32, tag="masks")
            nc.gpsimd.memset(masks[:], 1.0)
            for j in range(4):
                nc.gpsimd.affine_select(
                    out=masks[:, j * QC:(j + 1) * QC],
                    in_=masks[:, j * QC:(j + 1) * QC],
                    compare_op=ALU.is_ge, fill=0.0, base=-j * P,
                    pattern=[[1, QC]], channel_multiplier=-1,
                )
            b1_sb = cp.tile([P, NFC], f32, tag="b1_sb")
            nc.sync.dma_start(b1_sb[:], b1[:].rearrange("a p -> p a"))
            b2_sb = cp.tile([P, NDC], f32, tag="b2_sb")
            nc.sync.dma_start(b2_sb[:], b2[:].rearrange("a p -> p a"))
            esel_bc = cp.tile([P, E], f32, tag="esel_bc")
            nc.sync.dma_start(esel_bc[:], esel[0:1, :].to_broadcast((P, E)))

            # attention weights, resident
            wq_sb = cp.tile([P, NDC * HCOL], f32r, tag="wq_sb")
            wk_sb = cp.tile([P, NDC * HCOL], f32r, tag="wk_sb")
            wv_sb = cp.tile([P, NDC * HCOL], f32r, tag="wv_sb")
            wo_sb = cp.tile([P, D], f32r, tag="wo_sb")
            rw_sb = cp.tile([P, NDC * E], f32r, tag="rw_sb")
            lgT = cp.tile([E, T], f32r, tag="lgT")
            for w_sb, w_dr in ((wq_sb, wq), (wk_sb, wk), (wv_sb, wv)):
                nc.sync.dma_start(
                    w_sb[:], w_dr[:].rearrange("(a p) m -> p a m", p=P).bitcast(f32r)
                )
            nc.sync.dma_start(wo_sb[:], wo[:].bitcast(f32r))
            nc.sync.dma_start(
                rw_sb[:], rw[:].rearrange("(a p) m -> p a m", p=P).bitcast(f32r)
            )

            # ---- DRAM scratch ----
            r1_dram = dp.tile([1, T], f32, tag="r1_dram")
            r2_dram = dp.tile([1, T], f32, tag="r2_dram")
            ge_dram = dp.tile([1, T], f32, tag="ge_dram")
            x1T_dram = dp.tile([D, T], f32, tag="x1T_dram")
            ar_in = [dp.tile([D, ACW], f32, tag=f"ar_in{i}", name=f"ar_in{i}") for i in range(ACH)]
            ar_out = [dp.tile([D, ACW], f32, tag=f"ar_out{i}", name=f"ar_out{i}", addr_space="Shared") for i in range(ACH)]
            z_in = [dp.tile([D, ZW // 2], f32, tag=f"z_in{i}", name=f"z_in{i}") for i in range(2 * ZC)]
            z_out = [dp.tile([D, ZW // 2], f32, tag=f"z_out{i}", name=f"z_out{i}", addr_space="Shared") for i in range(2 * ZC)]

            # ================= phase A: r1 = rsqrt(mean(x^2)+eps) ============
            with (
                tc.tile_pool(name="pa", bufs=4) as pa,
                tc.tile_pool(name="pa1", bufs=4) as pa1,
            ):
                for tt in range(T // P):
                    xt = pa.tile([P, D], f32, tag="xt")
                    nc.sync.dma_start(xt[:], x_nat[tt * P:(tt + 1) * P, :])
                    sq = pa.tile([P, D], f32, tag="sq")
                    ss = pa1.tile([P, 1], f32, tag="ss")
                    nc.scalar.activation(sq[:], xt[:], ACT.Square, accum_out=ss[:])
                    ms = pa1.tile([P, 1], f32, tag="ms")
                    nc.vector.tensor_scalar(ms[:], ss[:], 1.0 / D, EPS,
                                            op0=ALU.mult, op1=ALU.add)
                    sr = pa1.tile([P, 1], f32, tag="sr")
                    nc.scalar.sqrt(sr[:], ms[:])
                    r1t = pa1.tile([P, 1], f32, tag="r1t")
                    nc.vector.reciprocal(r1t[:], sr[:])
                    nc.sync.dma_start(
                        r1_dram[0:1, tt * P:(tt + 1) * P].rearrange("o p -> p o"),
                        r1t[:],
                    )

            # ================= phase B/C: attention ==========================
            with (
                tc.tile_pool(name="attn", bufs=1) as ap,      # persistent
                tc.tile_pool(name="proj", bufs=4) as pj,      # streamed
                tc.tile_pool(name="projp", bufs=2, space="PSUM") as pjp,
            ):
                qT = ap.tile([P, T], f32r, tag="qT")
                kT = ap.tile([P, T], f32r, tag="kT")
                # v_aug: per (b, h, kt): [P, 65] block, col 64 == 1.0
                v_aug = ap.tile([P, B * HPC * NKT * 65], f32r, tag="v_aug")
                ctxn = ap.tile([P, T], f32r, tag="ctxn")

                # --- projections ---
                for w_sb, dstT in ((wq_sb, qT), (wk_sb, kT), (wv_sb, None)):
                    for tch in range(T // QC):
                        sl = slice(tch * QC, (tch + 1) * QC)
                        ps = pjp.tile([P, QC], f32, tag="proj_ps")
                        for dc in range(NDC):
                            xt = pj.tile([P, QC], f32r, tag="xtile")
                            nc.sync.dma_start(
                                xt[:], xT[dc * P:(dc + 1) * P, sl].bitcast(f32r)
                            )
                            nc.tensor.matmul(
                                ps[:], w_sb[:, dc * HCOL:(dc + 1) * HCOL], xt[:],
                                start=(dc == 0), stop=(dc == NDC - 1),
                            )
                        r1bc = pj.tile([P, QC], f32, tag="r1bc")
                        nc.sync.dma_start(r1bc[:], r1_dram[0:1, sl].to_broadcast((P, QC)))
                        if dstT is not None:
                            nc.vector.tensor_mul(dstT[:, sl], ps[:], r1bc[:])
                        else:
                            vts = pj.tile([P, QC], f32r, tag="vts")
                            nc.vector.tensor_mul(vts[:], ps[:], r1bc[:])
                            # transpose 128-blocks into v_aug
                            b_ = tch // NQC
                            for blk in range(QC // P):
                                kt_ = (tch % NQC) * (QC // P) + blk
                                vtp = pjp.tile([P, P], f32r, tag="vt_ps", bufs=1)
                                nc.tensor.transpose(
                                    vtp[:], vts[:, blk * P:(blk + 1) * P], identr[:]
                                )
                                for h in range(HPC):
                                    idx = ((b_ * HPC + h) * NKT + kt_) * 65
                                    nc.scalar.copy(
                                        v_aug[:, idx:idx + HD],
                                        vtp[:, h * HD:(h + 1) * HD],
                                    )
                if debug_taps:
                    tq = pj.tile([P, T], f32, tag="tapq")
                    nc.vector.tensor_copy(tq[:], qT[:])
                    nc.sync.dma_start(taps["qT"][:], tq[:])
                    tk = pj.tile([P, T], f32, tag="tapk")
                    nc.vector.tensor_copy(tk[:], kT[:])
                    nc.sync.dma_start(taps["kT"][:], tk[:])

                # --- scores / softmax / context ---
                with (
                    tc.tile_pool(name="sc", bufs=4) as scp,
                    tc.tile_pool(name="scps", bufs=3, space="PSUM") as scps,
                    tc.tile_pool(name="ctxps", bufs=1, space="PSUM") as ctxps,
                ):
                    for b_ in range(B):
                        for qc_ in range(NQC):
                            qsl = slice(b_ * S + qc_ * QC, b_ * S + (qc_ + 1) * QC)
                            nkt = (qc_ + 1) * (QC // P)
                            cps = [
                                ctxps.tile([65, QC], f32, tag=f"ctx_ps{h}",
                                           name=f"ctx_ps{h}")
                                for h in range(HPC)
                            ]
                            for kt_ in range(nkt):
                                for h in range(HPC):
                                    hsl = slice(h * HD, (h + 1) * HD)
                                    ksl = slice(b_ * S + kt_ * P, b_ * S + (kt_ + 1) * P)
                                    sps = scps.tile([P, QC], f32, tag="s_ps")
                                    nc.tensor.matmul(
                                        sps[:], kT[hsl, ksl], qT[hsl, qsl],
                                        start=True, stop=True,
                                    )
                                    ex = scp.tile([P, QC], f32r, tag="ex")
                                    nc.scalar.activation(ex[:], sps[:], ACT.Exp)
                                    j = kt_ - (qc_ * (QC // P))
                                    if j >= 0:
                                        nc.vector.tensor_mul(
                                            ex[:], ex[:], masks[:, j * QC:(j + 1) * QC]
                                        )
                                    idx = ((b_ * HPC + h) * NKT + kt_) * 65
                                    nc.tensor.matmul(
                                        cps[h][:], v_aug[:, idx:idx + 65], ex[:],
                                        start=(kt_ == 0), stop=(kt_ == nkt - 1),
                                    )
                            # normalize: ctxn[h*64:(h+1)*64, qsl] = ctx / sumexp
                            for h in range(HPC):
                                rec = scp.tile([1, QC], f32r, tag="rec")
                                with nc.allow_low_precision(reason="f32r softmax recip"):
                                    nc.vector.reciprocal(rec[:], cps[h][64:65, :])
                                bc = scps.tile([HD, QC], f32, tag="bc_ps")
                                nc.tensor.matmul(
                                    bc[:], ones_r[0:1, 0:HD], rec[:],
                                    start=True, stop=True,
                                )
                                bcs = scp.tile([HD, QC], f32, tag="bcs")
                                nc.vector.tensor_copy(bcs[:], bc[:])
                                nc.vector.tensor_mul(
                                    ctxn[h * HD:(h + 1) * HD, qsl],
                                    cps[h][0:HD, :], bcs[:],
                                )
                if debug_taps:
                    tcx = pj.tile([P, T], f32, tag="tapcx")
                    nc.vector.tensor_copy(tcx[:], ctxn[:])
                    nc.sync.dma_start(taps["ctxn"][:], tcx[:])

                # --- output projection partials + chunked AllReduce ---
                with (
                    tc.tile_pool(name="wop", bufs=4) as wop,
                    tc.tile_pool(name="wops", bufs=3, space="PSUM") as wops,
                ):
                    for ch in range(ACH):
                        for tch in range(ACW // QC):
                            sl = slice(ch * ACW + tch * QC, ch * ACW + (tch + 1) * QC)
                            ot = wop.tile([P, NDC * QC], f32, tag="wo_sb_t", bufs=2)
                            for dc in range(NDC):
                                ps = wops.tile([P, QC], f32, tag="wo_ps")
                                nc.tensor.matmul(
                                    ps[:], wo_sb[:, dc * P:(dc + 1) * P], ctxn[:, sl],
                                    start=True, stop=True,
                                )
                                nc.vector.tensor_copy(ot[:, dc * QC:(dc + 1) * QC], ps[:])
                            nc.sync.dma_start(
                                ar_in[ch][:, tch * QC:(tch + 1) * QC].rearrange(
                                    "(a p) t -> p a t", p=P),
                                ot[:],
                            )
                        all_reduce(ar_in[ch], ar_out[ch])
                        # x1 = x + attn_out for this chunk (overlaps next chunk)
                        AQ = ACW // 4
                        ssrow = wop.tile([1, ACW], f32, tag="ssrow", bufs=1)
                        for qtr in range(4):
                            xtc = wop.tile([P, NDC * AQ], f32, tag="xtc", bufs=1)
                            arc = wop.tile([P, NDC * AQ], f32, tag="arc", bufs=1)
                            x1c = wop.tile([P, NDC * AQ], f32r, tag="x1c", bufs=1)
                            hsl2 = slice(ch * ACW + qtr * AQ,
                                         ch * ACW + (qtr + 1) * AQ)
                            nc.sync.dma_start(
                                xtc[:],
                                xT[:, hsl2].rearrange("(a p) t -> p a t", p=P))
                            nc.sync.dma_start(
                                arc[:],
                                ar_out[ch][:, qtr * AQ:(qtr + 1) * AQ].rearrange(
                                    "(a p) t -> p a t", p=P))
                            nc.vector.tensor_add(x1c[:], xtc[:], arc[:])
                            nc.sync.dma_start(
                                x1T_dram[:, hsl2].rearrange(
                                    "(a p) t -> p a t", p=P).bitcast(f32r),
                                x1c[:])
                            # fused router logits + sumsq for this quarter
                            sqc = wop.tile([P, NDC * AQ], f32r, tag="sqc", bufs=1)
                            nc.scalar.activation(sqc[:], x1c[:], ACT.Square)
                            lg_ps = wops.tile([E, AQ], f32, tag="lg_ps")
                            ss_ps = wops.tile([1, AQ], f32, tag="ss_ps")
                            for dc in range(NDC):
                                st_ = (dc == 0)
                                sp_ = (dc == NDC - 1)
                                nc.tensor.matmul(
                                    lg_ps[:], rw_sb[:, dc * E:(dc + 1) * E],
                                    x1c[:, dc * AQ:(dc + 1) * AQ],
                                    start=st_, stop=sp_)
                                nc.tensor.matmul(
                                    ss_ps[:], ones_r[:, 0:1],
                                    sqc[:, dc * AQ:(dc + 1) * AQ],
                                    start=st_, stop=sp_)
                            nc.vector.tensor_copy(lgT[:, hsl2], lg_ps[:])
                            nc.vector.tensor_scalar(
                                ssrow[:, qtr * AQ:(qtr + 1) * AQ], ss_ps[:],
                                1.0 / D, EPS, op0=ALU.mult, op1=ALU.add)
                        srq = wop.tile([1, ACW], f32, tag="srq", bufs=1)
                        nc.scalar.sqrt(srq[:], ssrow[:])
                        r2q = wop.tile([1, ACW], f32, tag="r2q", bufs=1)
                        nc.vector.reciprocal(r2q[:], srq[:])
                        nc.sync.dma_start(
                            r2_dram[0:1, ch * ACW:(ch + 1) * ACW], r2q[:])

            # ================= phase D: gates =================================
            with tc.tile_pool(name="gt", bufs=1) as gt:
              # scale logits by r2 (per token, along free axis)
                r2bc8 = gt.tile([E, T], f32, tag="r2bc8")
                nc.sync.dma_start(r2bc8[:], r2_dram[0:1, :].to_broadcast((E, T)))
                nc.vector.tensor_mul(lgT[:], lgT[:], r2bc8[:])
                if debug_taps:
                    nc.sync.dma_start(taps["r2"][:], r2_dram[0:1, :])

                # transpose logits to [token, E]; top-2 gates
                with (
                    tc.tile_pool(name="g2", bufs=4) as g2,
                    tc.tile_pool(name="g2ps", bufs=4, space="PSUM") as g2ps,
                ):
                    for tt in range(T // P):
                        lp = g2ps.tile([P, E], f32r, tag="lg_t_ps")
                        nc.tensor.transpose(
                            lp[:], lgT[:, tt * P:(tt + 1) * P].bitcast(f32r), identr[:]
                        )
                        lg = g2.tile([P, E], f32, tag="lg")
                        nc.scalar.copy(lg[:], lp[:])
                        m1 = g2.tile([P, 1], f32, tag="m1")
                        nc.vector.tensor_reduce(m1[:], lg[:], axis=AX.X, op=ALU.max)
                        mk1 = g2.tile([P, E], f32, tag="mk1")
                        nc.vector.tensor_scalar(mk1[:], lg[:], m1[:], None,
                                                op0=ALU.is_equal)
                        msk = g2.tile([P, E], f32, tag="msk")
                        nc.vector.scalar_tensor_tensor(
                            msk[:], mk1[:], -1e30, lg[:], op0=ALU.mult, op1=ALU.add
                        )
                        m2 = g2.tile([P, 1], f32, tag="m2")
                        nc.vector.tensor_reduce(m2[:], msk[:], axis=AX.X, op=ALU.max)
                        mk2 = g2.tile([P, E], f32, tag="mk2")
                        nc.vector.tensor_scalar(mk2[:], msk[:], m2[:], None,
                                                op0=ALU.is_equal)
                        dlt = g2.tile([P, 1], f32, tag="dlt")
                        nc.vector.tensor_sub(dlt[:], m2[:], m1[:])
                        g1 = g2.tile([P, 1], f32, tag="g1")
                        nc.scalar.activation(g1[:], dlt[:], ACT.Sigmoid, scale=-1.0)
                        g2_ = g2.tile([P, 1], f32, tag="g2_")
                        nc.vector.tensor_scalar(g2_[:], g1[:], -1.0, 1.0,
                                                op0=ALU.mult, op1=ALU.add)
                        gts = g2.tile([P, E], f32, tag="gts")
                        nc.vector.tensor_scalar(gts[:], mk1[:], g1[:], None,
                                                op0=ALU.mult)
                        nc.vector.scalar_tensor_tensor(
                            gts[:], mk2[:], g2_[:], gts[:], op0=ALU.mult, op1=ALU.add
                        )
                        if debug_taps:
                            nc.sync.dma_start(
                                taps["logits"][tt * P:(tt + 1) * P, :], lg[:]
                            )
                            nc.sync.dma_start(
                                taps["gates"][tt * P:(tt + 1) * P, :], gts[:]
                            )
                        # my expert's gate column -> staged [P, 32] tile
                        gsel = g2.tile([P, E], f32, tag="gsel")
                        nc.vector.tensor_mul(gsel[:], gts[:], esel_bc[:])
                        nc.vector.tensor_reduce(gcols[:, tt:tt + 1], gsel[:],
                                                axis=AX.X, op=ALU.add)
                if debug_taps:
                    xx = rt.tile([P, T], f32, tag="tapx1")
                    for dc in range(NDC):
                        nc.sync.dma_start(xx[:], x1T_dram[dc * P:(dc + 1) * P, :])
                        nc.sync.dma_start(taps["x1T"][dc * P:(dc + 1) * P, :], xx[:])

            # ================= phase E: expert MLP + combine =================
            # Token halves (ZC); within a half, fc-groups of GFC so each
            # weight tile is loaded once per half and amortized over all
            # NTC token chunks.  y accumulates in SBUF (bf16).
            NTC = ZW // QC  # token chunks of 512 per half
            with (
                tc.tile_pool(name="mo", bufs=1) as mo,
                tc.tile_pool(name="mow", bufs=8) as mow,
                tc.tile_pool(name="moz", bufs=2) as moz,
                tc.tile_pool(name="mops", bufs=2, space="PSUM") as mops,
            ):
                for zc in range(ZC):
                    zsl = slice(zc * ZW, (zc + 1) * ZW)
                    h2r = mo.tile([P, NDC * ZW], bf16, tag="h2r", bufs=2)
                    ysb = mo.tile([P, NDC * ZW], bf16, tag="ysb")
                    eh = [
                        mo.tile([P, GFC * ZW], bf16, tag=f"eh{i}", name=f"eh{i}")
                        for i in range(2)
                    ]
                    r2bc = moz.tile([P, ZW], f32, tag="r2bc")
                    nc.sync.dma_start(r2bc[:], r2_dram[0:1, zsl].to_broadcast((P, ZW)))
                    gebc = moz.tile([P, ZW], f32, tag="gebc")
                    nc.sync.dma_start(gebc[:], ge_dram[0:1, zsl].to_broadcast((P, ZW)))
                    for dc in range(NDC):
                        x1s = moz.tile([P, ZW], f32, tag="x1s")
                        nc.sync.dma_start(x1s[:],
                                          x1T_dram[dc * P:(dc + 1) * P, zsl])
                        nc.vector.tensor_mul(h2r[:, dc * ZW:(dc + 1) * ZW],
                                             x1s[:], r2bc[:])
                    for g in range(NFC // GFC):
                        ehg = eh[g % 2]
                        # --- w1 stage: eh_g = gelu(w1_g.T @ h2 + b1) ---
                        for gi in range(GFC):
                            fc = g * GFC + gi
                            pss = [
                                mops.tile([P, QC], f32, tag=f"s_ps{t}",
                                          name=f"s_ps{t}")
                                for t in range(NTC)
                            ]
                            wt = mow.tile([P, NDC * P], bf16, tag="w1tile")
                            nc.sync.dma_start(wt[:], w1t[fc])
                            for dc in range(NDC):
                                for t in range(NTC):
                                    nc.tensor.matmul(
                                        pss[t][:], wt[:, dc * P:(dc + 1) * P],
                                        h2r[:, dc * ZW + t * QC:
                                            dc * ZW + (t + 1) * QC],
                                        start=(dc == 0), stop=(dc == NDC - 1))
                            for t in range(NTC):
                                nc.scalar.activation(
                                    ehg[:, gi * ZW + t * QC: gi * ZW + (t + 1) * QC],
                                    pss[t][:],
                                    ACT.Gelu_apprx_tanh, bias=b1_sb[:, fc:fc + 1])
                        # --- w2 stage: y += w2_g.T @ eh_g ---
                        for dc in range(NDC):
                            pss = [
                                mops.tile([P, QC], f32, tag=f"s_ps{t}",
                                          name=f"s_ps{t}")
                                for t in range(NTC)
                            ]
                            wt = mow.tile([P, GFC * P], bf16, tag="w2tile")
                            nc.sync.dma_start(wt[:], w2t[dc, g])
                            for gi in range(GFC):
                                for t in range(NTC):
                                    nc.tensor.matmul(
                                        pss[t][:], wt[:, gi * P:(gi + 1) * P],
                                        ehg[:, gi * ZW + t * QC:
                                            gi * ZW + (t + 1) * QC],
                                        start=(gi == 0), stop=(gi == GFC - 1))
                            for t in range(NTC):
                                ysl = slice(dc * ZW + t * QC, dc * ZW + (t + 1) * QC)
                                if g == 0:
                                    nc.vector.tensor_copy(ysb[:, ysl], pss[t][:])
                                else:
                                    nc.vector.tensor_add(ysb[:, ysl], pss[t][:],
                                                         ysb[:, ysl])
                    # --- combine: z = (y + b2) * gate + x1/8, then AllReduce ---
                    for dc in range(NDC):
                        x1s2 = moz.tile([P, ZW], f32, tag="x1s2")
                        nc.sync.dma_start(x1s2[:],
                                          x1T_dram[dc * P:(dc + 1) * P, zsl])
                        t1 = moz.tile([P, ZW], f32, tag="t1")
                        nc.vector.scalar_tensor_tensor(
                            t1[:], ysb[:, dc * ZW:(dc + 1) * ZW],
                            b2_sb[:, dc:dc + 1], gebc[:],
                            op0=ALU.add, op1=ALU.mult)
                        zt = moz.tile([P, ZW], f32, tag="zt")
                        nc.vector.scalar_tensor_tensor(
                            zt[:], x1s2[:], 1.0 / N_CORES, t1[:],
                            op0=ALU.mult, op1=ALU.add)
                        for half in range(2):
                            nc.sync.dma_start(
                                z_in[2 * zc + half][dc * P:(dc + 1) * P, :],
                                zt[:, half * (ZW // 2):(half + 1) * (ZW // 2)])
                    for half in range(2):
                        hw2 = ZW // 2
                        all_reduce(z_in[2 * zc + half], z_out[2 * zc + half])
                        nc.sync.dma_start(
                            outT[:, zc * ZW + half * hw2: zc * ZW + (half + 1) * hw2],
                            z_out[2 * zc + half][:])

    nc.compile()
    _NC_CACHE[key] = nc
    return nc


def make_in_maps(x, n1_w, n2_w, wq, wk, wv, wo, router_w, w1, b1, w2, b2):
    x = np.asarray(x, np.float32)
    x2 = x.reshape(T, D)
    xT = np.ascontiguousarray(x2.T)
    n1 = np.asarray(n1_w, np.float32)
    n2 = np.asarray(n2_w, np.float32)
    wq_e = (n1[:, None] * np.asarray(wq, np.float32)) * (HD ** -0.5)
    wk_e = n1[:, None] * np.asarray(wk, np.float32)
    wv_e = n1[:, None] * np.asarray(wv, np.float32)
    rw_e = np.ascontiguousarray((np.asarray(router_w, np.float32) * n2[None, :]).T)
    in_maps = []
    for c in range(N_CORES):
        cols = slice(c * HCOL, (c + 1) * HCOL)
        w1_e = n2[:, None] * np.asarray(w1[c], np.float32)          # [D, F]
        w1t = np.ascontiguousarray(
            w1_e.reshape(NDC, P, NFC, P).transpose(2, 1, 0, 3).reshape(NFC, P, NDC * P)
        ).astype(ml_dtypes.bfloat16)
        w2_c = np.asarray(w2[c], np.float32)                        # [F, D]
        w2t = np.ascontiguousarray(
            w2_c.reshape(NGRP, GFC, P, NDC, P).transpose(3, 0, 2, 1, 4).reshape(
                NDC, NGRP, P, GFC * P)
        ).astype(ml_dtypes.bfloat16)
        esel = np.zeros((1, E), np.float32)
        esel[0, c] = 1.0
        in_maps.append({
            "xT": xT,
            "wq": np.ascontiguousarray(wq_e[:, cols]),
            "wk": np.ascontiguousarray(wk_e[:, cols]),
            "wv": np.ascontiguousarray(wv_e[:, cols]),
            "wo": np.ascontiguousarray(np.asarray(wo, np.float32)[cols, :]),
            "rw": rw_e,
            "w1t": w1t,
            "w2t": w2t,
            "b1": np.ascontiguousarray(np.asarray(b1[c], np.float32).reshape(NFC, P)),
            "b2": np.ascontiguousarray(np.asarray(b2[c], np.float32).reshape(NDC, P)),
            "esel": esel,
        })
    return in_maps


def kernel(**inputs) -> np.ndarray:
    nc = build_nc()
    in_maps = make_in_maps(**inputs)
    res = run_bass_kernel_spmd(nc, in_maps, core_ids=list(range(N_CORES)),
                               trace=False)
    outT = res.results[0]["outT"]
    return np.ascontiguousarray(outT.T).reshape(B, S, D)
